# revision 27
# baseline (speedup 1.0000x reference)
"""Block-circulant linear layer on 8 Trainium2 NeuronCores.

Math: y[n, (j,b)] = sum_i circconv(x[n,i,:], c[j,i,:])[b] + bias.
Factorized via packed-real-FFT (halfcomplex, 128 slots of (re,im)):

  stage A (rfft):    t1 = F_pack^T @ x^T     per in-block i, slot-partition out
  permute A->B:      slot-group regroup (32 DMA row shuffles / chunk)
  stage B (mixing):  t2 = W2_g^T @ t1p       block-diagonal per slot-group g
  permute B->C:      slot-major regroup (32 DMAs / chunk)
  stage C (irfft):   y[b, t] = G_slice^T @ t2p  (feature-major out; G stationary)
  bias:              fused into the PSUM->SBUF evacuation (per-partition scalar)

All matmuls bf16 with N=chunk moving columns; PSUM fp32. Stages are
software-pipelined across chunks: A(t) | B(t-1) | C(t-2).
Sharding: data-parallel, 1024 tokens per core; weights replicated.
Host side: transpose/pack x shards, build F/W2/G/biasfm, unpack y (layout only).
"""

import numpy as np

try:
    import ml_dtypes
    _BF16 = ml_dtypes.bfloat16
except ImportError:  # pragma: no cover
    _BF16 = None

MID_BF16 = True
CHUNK = 512      # tokens per pipeline chunk (256 or 512)

BLOCK = 256
NB = 16          # in/out blocks
NSLOT = 128      # frequency slots (halfcomplex pairs)
N_CORES = 8
TOK_PER_CORE = 1024
IN_F = NB * BLOCK  # 4096


def _build_weights(c: np.ndarray):
    """Host-side weight construction (float64 for accuracy, cast to f32)."""
    B, K = BLOCK, NSLOT
    b = np.arange(B)
    k = np.arange(K)
    theta = 2 * np.pi * np.outer(b, k) / B
    F_re = np.cos(theta)
    F_im = -np.sin(theta)
    F_im[:, 0] = (-1.0) ** b            # Nyquist column in the c=1 half, k=0
    F_pack = np.concatenate([F_re, F_im], axis=1)   # [256 b, 256 (c,k)]

    G_re = np.zeros((K, B))
    G_im = np.zeros((K, B))
    kk = np.arange(1, K)
    th = 2 * np.pi * np.outer(kk, b) / B
    G_re[1:] = 2.0 * np.cos(th) / B
    G_re[0] = 1.0 / B
    G_im[1:] = -2.0 * np.sin(th) / B
    G_im[0] = ((-1.0) ** b) / B
    G_pack = np.stack([G_re, G_im], axis=0)          # [2, 128 k, 256 b]

    Cf = np.fft.rfft(c.astype(np.float64), axis=-1)  # [j, i, 129]
    A = Cf.real
    Bm = Cf.imag
    W2 = np.zeros((32, 128, 128))
    for g in range(32):
        for s in range(4):
            ks = 4 * g + s
            blk = np.zeros((32, 32))                 # rows (c,i) -> cols (c',j)
            if ks == 0:
                blk[0:16, 0:16] = A[:, :, 0].T
                blk[16:32, 16:32] = A[:, :, 128].T
            else:
                a = A[:, :, ks].T
                bb = Bm[:, :, ks].T
                blk[0:16, 0:16] = a
                blk[16:32, 0:16] = -bb
                blk[0:16, 16:32] = bb
                blk[16:32, 16:32] = a
            W2[g, 32 * s:32 * s + 32, 32 * s:32 * s + 32] = blk

    f_host = (
        F_pack.reshape(2, 128, 2, 128).transpose(1, 0, 2, 3).reshape(128, 512)
    )  # [p=b_local, bh*256 + ch*128 + k]
    w2_host = W2.transpose(1, 0, 2).reshape(128, 32 * 128)   # [p, 128g + m]
    g_host = G_pack.transpose(1, 0, 2).reshape(128, 512)     # [k, ch*256 + b]
    return (
        f_host.astype(np.float32),
        w2_host.astype(np.float32),
        g_host.astype(np.float32),
    )


_NC_CACHE = {}


def _build_module(mid_bf16=True, chunk=CHUNK, pin_bufs=None, mid_bufs=None,
                  psum_bufs=(2, 2, 4), perm_engines="3way",
                  io_eng_name="gpsimd", tick_order=None, sw_pipe=True,
                  repeat=1, skip_perm=False, skip_store=False):
    """Build + compile the per-core Bass module (cached)."""
    CK = chunk
    NCH = TOK_PER_CORE // CK          # chunks per core
    n_pieces = (32 * CK) // 8192      # load pieces per chunk (1 or 2)
    if pin_bufs is None:
        pin_bufs = 2
    if mid_bufs is None:
        mid_bufs = 1 if CK == 512 else 2
    if tick_order is None:
        tick_order = "acb" if CK == 512 else "abc"
    psum_bufs = tuple(psum_bufs)
    key = ("nc4", mid_bf16, CK, pin_bufs, mid_bufs, psum_bufs, perm_engines,
           io_eng_name, tick_order, sw_pipe, repeat, skip_perm, skip_store)
    if key in _NC_CACHE:
        return _NC_CACHE[key]

    import concourse.bass as bass  # noqa: F401
    import concourse.mybir as mybir
    import concourse.tile as tile
    from concourse import bacc

    f32 = mybir.dt.float32
    bf16 = mybir.dt.bfloat16
    mid_dt = bf16 if mid_bf16 else f32
    Identity = mybir.ActivationFunctionType.Identity

    nc = bacc.Bacc("TRN2", target_bir_lowering=False, debug=False)

    xt_d = nc.dram_tensor(
        "xt", [NCH, 128, 32, CK], mid_dt, kind="ExternalInput"
    )
    f_d = nc.dram_tensor("fw", [128, 512], mid_dt, kind="ExternalInput")
    w2_d = nc.dram_tensor("w2", [128, 4096], mid_dt, kind="ExternalInput")
    g_d = nc.dram_tensor("gw", [128, 512], mid_dt, kind="ExternalInput")
    bias_d = nc.dram_tensor("biasfm", [128, 32], f32, kind="ExternalInput")
    y_d = nc.dram_tensor(
        "y", [NCH, 128, 32, CK], mid_dt, kind="ExternalOutput"
    )

    perm_n = [0]

    MIDW = 32 * CK                    # mid tile width (elements per partition)
    n_store = MIDW // 4096            # ysb store pieces per chunk (2 or 4)
    reg_per_store = 4096 // CK        # (2j+bh) regions per ysb piece
    ck_per_bank = 512 // CK           # chunk-cols per PSUM bank (2 or 1)
    jg = max(1, psum_bufs[2] * ck_per_bank // 2)   # j's in flight in stage C

    def perm_eng():
        engs = {
            "ss": (nc.sync, nc.scalar),
            "3way": (nc.sync, nc.scalar, nc.gpsimd),
            "sg": (nc.sync, nc.gpsimd),
            "s": (nc.sync,),
            "gp": (nc.gpsimd,),
        }[perm_engines]
        e = engs[perm_n[0] % len(engs)]
        perm_n[0] += 1
        return e

    io_engine = {"gpsimd": "gpsimd", "scalar": "scalar", "sync": "sync"}[io_eng_name]

    with tile.TileContext(nc) as tc:
        with (
            tc.tile_pool(name="wpool", bufs=1) as wpool,
            tc.tile_pool(name="pin", bufs=pin_bufs) as pin,
            tc.tile_pool(name="pt1", bufs=mid_bufs) as pt1,
            tc.tile_pool(name="pt1p", bufs=mid_bufs) as pt1p,
            tc.tile_pool(name="pt2", bufs=mid_bufs) as pt2,
            tc.tile_pool(name="pt2p", bufs=mid_bufs) as pt2p,
            tc.tile_pool(name="pysb", bufs=2) as pysb,
            tc.tile_pool(name="psA", bufs=psum_bufs[0], space="PSUM") as psA,
            tc.tile_pool(name="psB", bufs=psum_bufs[1], space="PSUM") as psB,
            tc.tile_pool(name="psC", bufs=psum_bufs[2], space="PSUM") as psC,
        ):
            f_sb = wpool.tile([128, 512], mid_dt, tag="fw")
            w2_sb = wpool.tile([128, 4096], mid_dt, tag="w2")
            g_sb = wpool.tile([128, 512], mid_dt, tag="gw")
            bias_sb = wpool.tile([128, 32], f32, tag="biasfm")
            nc.sync.dma_start(out=f_sb[:], in_=f_d[:])
            nc.sync.dma_start(out=w2_sb[:], in_=w2_d[:])
            nc.sync.dma_start(out=g_sb[:], in_=g_d[:])
            nc.sync.dma_start(out=bias_sb[:], in_=bias_d[:])

            evac_n = [0]

            def evac(dst, srcp):
                if evac_n[0] % 2 == 0:
                    nc.vector.tensor_copy(dst, srcp)
                else:
                    nc.scalar.copy(dst, srcp)
                evac_n[0] += 1

            def evac_bias(dst, srcp, bias_ap):
                if evac_n[0] % 2 == 0:
                    nc.vector.tensor_scalar_add(dst, srcp, bias_ap)
                else:
                    nc.scalar.activation(dst, srcp, Identity, bias=bias_ap)
                evac_n[0] += 1

            def io(eng_name):
                return getattr(nc, eng_name)

            xts_t = {}   # (ci, piece) -> tile
            t1p_t = {}
            t2p_t = {}

            def do_load(ci, piece=None):
                pieces = range(n_pieces) if piece is None else [piece]
                for h in pieces:
                    xts = pin.tile([128, 8192], mid_dt, tag="pin")
                    fpp = 32 // n_pieces   # f-blocks per piece
                    io(io_engine).dma_start(
                        out=xts[:].rearrange("p (f t) -> p f t", f=fpp),
                        in_=xt_d[ci % NCH, :, h * fpp: (h + 1) * fpp],
                    )
                    xts_t[(ci, h)] = xts

            def do_A(ci, prefetch=None):
                # stage A: rfft per in-block; out t1[k, (16ch+i)*CK + t]
                t1 = pt1.tile([128, MIDW], mid_dt, tag="t1")
                fpp = 32 // n_pieces
                loaded = set()
                # ii-major so load piece h is released after its ii range
                for iq in range(0, NB, ck_per_bank):
                    for ch in range(2):
                        ps = psA.tile([128, 512], f32, tag="psA")
                        n_mm = 0
                        for ii in range(iq, iq + ck_per_bank):
                            for bh in range(2):
                                f = 2 * ii + bh
                                xts = xts_t[(ci, f // fpp)]
                                off = (ii - iq) * CK
                                nc.tensor.matmul(
                                    ps[:, off: off + CK],
                                    f_sb[:, bh * 256 + ch * 128:
                                         bh * 256 + ch * 128 + 128],
                                    xts[:, (f % fpp) * CK: (f % fpp) * CK + CK],
                                    start=(n_mm == 0),
                                    stop=(n_mm == 2 * ck_per_bank - 1),
                                )
                                n_mm += 1
                        q1 = 16 * ch + iq
                        evac(t1[:, q1 * CK: q1 * CK + ck_per_bank * CK], ps[:])
                    if prefetch is not None and n_pieces > 1:
                        # piece h of next chunk, once piece h of this one is done
                        h_done = (iq + ck_per_bank) * 2 // fpp - 1
                        if 0 <= h_done < n_pieces and h_done not in loaded:
                            loaded.add(h_done)
                            do_load(prefetch, piece=h_done)
                for h in range(n_pieces):
                    xts_t.pop((ci, h), None)
                if prefetch is not None and n_pieces == 1:
                    do_load(prefetch)
                # permute A->B: t1p[32s+q, g, t] = t1[4g+s, q, t]
                if skip_perm:
                    t1p_t[ci] = t1
                    return
                t1p = pt1p.tile([128, MIDW], mid_dt, tag="t1p")
                t1v = t1[:].rearrange("p (q t) -> p q t", t=CK)
                for g in range(32):
                    perm_eng().dma_start(
                        out=t1p[:, g * CK: g * CK + CK],
                        in_=t1v[4 * g: 4 * g + 4],
                    )
                t1p_t[ci] = t1p

            def do_B(ci):
                # stage B: per-slot-group mixing; out t2[(32s+16c+j), g*CK+t]
                t1p = t1p_t.pop(ci)
                t2 = pt2.tile([128, MIDW], mid_dt, tag="t2")
                for g0 in range(0, 32, ck_per_bank):
                    ps = psB.tile([128, 512], f32, tag="psB")
                    for gg in range(g0, g0 + ck_per_bank):
                        off = (gg - g0) * CK
                        nc.tensor.matmul(
                            ps[:, off: off + CK],
                            w2_sb[:, gg * 128: gg * 128 + 128],
                            t1p[:, gg * CK: gg * CK + CK],
                            start=(gg == g0),
                            stop=(gg == g0 + ck_per_bank - 1),
                        )
                    evac(t2[:, g0 * CK: g0 * CK + ck_per_bank * CK], ps[:])
                # permute B->C: t2p[4g+s, 16c+j, t] = t2[32s+16c+j, g, t]
                if skip_perm:
                    t2p_t[ci] = t2
                    return
                t2p = pt2p.tile([128, MIDW], mid_dt, tag="t2p")
                t2pv = t2p[:].rearrange("p (q t) -> p q t", t=CK)
                for g in range(32):
                    perm_eng().dma_start(
                        out=t2pv[4 * g: 4 * g + 4],
                        in_=t2[:, g * CK: g * CK + CK],
                    )
                t2p_t[ci] = t2p

            def do_C(ci):
                # stage C: irfft, feature-major; G slices stationary.
                # region q = 2j+bh of width CK; ysb pieces of 4096 cols
                t2p = t2p_t.pop(ci)
                for piece in range(n_store):
                    ysb = pysb.tile([128, 4096], mid_dt, tag="ysb")
                    q0 = piece * reg_per_store
                    j0, j1 = q0 // 2, (q0 + reg_per_store) // 2
                    for jq in range(j0, j1, jg):
                        pss = []
                        for _ in range(jg * 2 // ck_per_bank):
                            psc = psC.tile([128, 512], f32, tag="psC")
                            pss.append(psc)

                        def bank(jj, bh):
                            idx = (jj - jq) * 2 + bh
                            return pss[idx // ck_per_bank], (idx % ck_per_bank) * CK

                        for bh in range(2):
                            for ch in range(2):
                                for jj in range(jq, jq + jg):
                                    ps, off = bank(jj, bh)
                                    nc.tensor.matmul(
                                        ps[:, off: off + CK],
                                        g_sb[:, ch * 256 + bh * 128:
                                             ch * 256 + bh * 128 + 128],
                                        t2p[:, (16 * ch + jj) * CK:
                                            (16 * ch + jj) * CK + CK],
                                        start=(ch == 0 and (bh == 0 or ck_per_bank == 1)),
                                        stop=(ch == 1 and (bh == 1 or ck_per_bank == 1)),
                                    )
                        for jj in range(jq, jq + jg):
                            for bh in range(2):
                                ps, off = bank(jj, bh)
                                q = 2 * jj + bh
                                evac_bias(
                                    ysb[:, (q - q0) * CK: (q - q0) * CK + CK],
                                    ps[:, off: off + CK],
                                    bias_sb[:, q: q + 1],
                                )
                    if skip_store:
                        continue
                    io(io_engine).dma_start(
                        out=y_d[ci % NCH, :,
                                piece * reg_per_store: (piece + 1) * reg_per_store],
                        in_=ysb[:].rearrange("p (q t) -> p q t", t=CK),
                    )

            NT = NCH * repeat
            do_load(0)
            if NT > 1 and n_pieces == 1:
                do_load(1)
            def _pref(t):
                nxt = t + (2 if n_pieces == 1 else 1)
                return nxt if nxt < NT else None

            if sw_pipe:
                for t in range(NT + 2):
                    if tick_order == "acb":
                        if t < NT:
                            do_A(t, prefetch=_pref(t))
                        if 0 <= t - 2 < NT:
                            do_C(t - 2)
                        if 0 <= t - 1 < NT:
                            do_B(t - 1)
                    else:
                        if t < NT:
                            do_A(t, prefetch=_pref(t))
                        if 0 <= t - 1 < NT:
                            do_B(t - 1)
                        if 0 <= t - 2 < NT:
                            do_C(t - 2)
            else:
                for t in range(NT):
                    do_A(t, prefetch=_pref(t))
                    do_B(t)
                    do_C(t)

    nc.compile()
    _NC_CACHE[key] = nc
    return nc


def prepare_inputs(x, c, bias, chunk=CHUNK):
    """Host-side prep: shard + pack x, build weights. Returns per-core in_maps."""
    batch, seq, in_f = x.shape
    n_tok = batch * seq
    nch = TOK_PER_CORE // chunk
    xf = np.ascontiguousarray(x.reshape(n_tok, in_f).astype(np.float32))

    f_host, w2_host, g_host = _build_weights(np.asarray(c, dtype=np.float32))
    bias = np.asarray(bias, dtype=np.float32)
    # biasfm[p, 2j+bh] = bias[j*256 + bh*128 + p]
    biasfm = np.ascontiguousarray(
        bias.reshape(NB, 2, 128).transpose(2, 0, 1).reshape(128, 32)
    )
    if MID_BF16:
        f_host = f_host.astype(_BF16)
        w2_host = w2_host.astype(_BF16)
        g_host = g_host.astype(_BF16)

    in_maps = []
    for core in range(N_CORES):
        shard = xf[core * TOK_PER_CORE:(core + 1) * TOK_PER_CORE]  # [1024, 4096]
        # xt[ci, p, f, t] = shard[ci*chunk + t, 128*f + p]
        xt = np.ascontiguousarray(
            shard.reshape(nch, chunk, 32, 128).transpose(0, 3, 2, 1)
        )
        if MID_BF16:
            xt = xt.astype(_BF16)
        in_maps.append(
            {
                "xt": xt,
                "fw": f_host,
                "w2": w2_host,
                "gw": g_host,
                "biasfm": biasfm,
            }
        )
    return in_maps


def postprocess(y_cores, chunk=CHUNK):
    """y_cores: list of per-core y arrays [NCH, 128, 32, chunk] ->
    full [8192, 4096] float32 (token-major)."""
    nch = TOK_PER_CORE // chunk
    outs = []
    for yc in y_cores:
        yc = np.asarray(yc, dtype=np.float32).reshape(nch, 128, NB, 2, chunk)
        # y[ci*chunk+t, j*256+bh*128+p] = yc[ci, p, j, bh, t]
        outs.append(
            yc.transpose(0, 4, 2, 3, 1).reshape(TOK_PER_CORE, IN_F)
        )
    return np.concatenate(outs, axis=0)


def kernel(x: np.ndarray, c: np.ndarray, bias: np.ndarray) -> np.ndarray:
    from concourse.bass_utils import run_bass_kernel_spmd

    batch, seq, in_f = x.shape
    in_maps = prepare_inputs(x, c, bias)
    nc = _build_module(mid_bf16=MID_BF16)
    res = run_bass_kernel_spmd(nc, in_maps, core_ids=list(range(N_CORES)))
    y = postprocess([r["y"] for r in res.results])
    return y.reshape(batch, seq, in_f).astype(x.dtype)


# revision 30
# speedup vs baseline: 1.9807x; 1.9807x over previous
"""Block-circulant linear layer on 8 Trainium2 NeuronCores.

Math: y[n, (j,b)] = sum_i circconv(x[n,i,:], c[j,i,:])[b] + bias.
Factorized via packed-real-FFT (halfcomplex, 128 slots of (re,im)):

  stage A (rfft):    t1 = F_pack^T @ x^T     per in-block i, slot-partition out
  permute A->B:      slot-group regroup (32 DMA row shuffles / chunk)
  stage B (mixing):  t2 = W2_g^T @ t1p       block-diagonal per slot-group g
  permute B->C:      slot-major regroup (32 DMAs / chunk)
  stage C (irfft):   y[b, t] = G_slice^T @ t2p  (feature-major out; G stationary)
  bias:              fused into the PSUM->SBUF evacuation (per-partition scalar)

All matmuls bf16 with N=chunk moving columns; PSUM fp32. Stages are
software-pipelined across chunks: A(t) | B(t-1) | C(t-2).
Sharding: data-parallel, 1024 tokens per core; weights replicated.
Host side: transpose/pack x shards, build F/W2/G/biasfm, unpack y (layout only).
"""

import numpy as np

try:
    import ml_dtypes
    _BF16 = ml_dtypes.bfloat16
except ImportError:  # pragma: no cover
    _BF16 = None

MID_BF16 = True
CHUNK = 512      # tokens per pipeline chunk (256 or 512)

BLOCK = 256
NB = 16          # in/out blocks
NSLOT = 128      # frequency slots (halfcomplex pairs)
N_CORES = 8
TOK_PER_CORE = 1024
IN_F = NB * BLOCK  # 4096


def _build_weights(c: np.ndarray):
    """Host-side weight construction (float64 for accuracy, cast to f32)."""
    B, K = BLOCK, NSLOT
    b = np.arange(B)
    k = np.arange(K)
    theta = 2 * np.pi * np.outer(b, k) / B
    F_re = np.cos(theta)
    F_im = -np.sin(theta)
    F_im[:, 0] = (-1.0) ** b            # Nyquist column in the c=1 half, k=0
    F_pack = np.concatenate([F_re, F_im], axis=1)   # [256 b, 256 (c,k)]

    G_re = np.zeros((K, B))
    G_im = np.zeros((K, B))
    kk = np.arange(1, K)
    th = 2 * np.pi * np.outer(kk, b) / B
    G_re[1:] = 2.0 * np.cos(th) / B
    G_re[0] = 1.0 / B
    G_im[1:] = -2.0 * np.sin(th) / B
    G_im[0] = ((-1.0) ** b) / B
    G_pack = np.stack([G_re, G_im], axis=0)          # [2, 128 k, 256 b]

    Cf = np.fft.rfft(c.astype(np.float64), axis=-1)  # [j, i, 129]
    A = Cf.real
    Bm = Cf.imag
    W2 = np.zeros((32, 128, 128))
    for g in range(32):
        for s in range(4):
            ks = 4 * g + s
            blk = np.zeros((32, 32))                 # rows (c,i) -> cols (c',j)
            if ks == 0:
                blk[0:16, 0:16] = A[:, :, 0].T
                blk[16:32, 16:32] = A[:, :, 128].T
            else:
                a = A[:, :, ks].T
                bb = Bm[:, :, ks].T
                blk[0:16, 0:16] = a
                blk[16:32, 0:16] = -bb
                blk[0:16, 16:32] = bb
                blk[16:32, 16:32] = a
            W2[g, 32 * s:32 * s + 32, 32 * s:32 * s + 32] = blk

    f_host = (
        F_pack.reshape(2, 128, 2, 128).transpose(1, 0, 2, 3).reshape(128, 512)
    )  # [p=b_local, bh*256 + ch*128 + k]
    w2_host = W2.transpose(1, 0, 2).reshape(128, 32 * 128)   # [p, 128g + m]
    g_host = G_pack.transpose(1, 0, 2).reshape(128, 512)     # [k, ch*256 + b]
    return (
        f_host.astype(np.float32),
        w2_host.astype(np.float32),
        g_host.astype(np.float32),
    )


_NC_CACHE = {}


def _build_module(mid_bf16=True, chunk=CHUNK, pin_bufs=None, mid_bufs=None,
                  psum_bufs=(2, 2, 4), perm_engines="3way",
                  io_eng_name="gpsimd", tick_order=None, sw_pipe=True,
                  repeat=1, skip_perm=False, skip_store=False):
    """Build + compile the per-core Bass module (cached)."""
    CK = chunk
    NCH = TOK_PER_CORE // CK          # chunks per core
    n_pieces = max(1, (32 * CK) // 8192)  # load pieces per chunk
    pin_w = min(8192, 32 * CK)        # pin tile width
    if pin_bufs is None:
        pin_bufs = 2
    if mid_bufs is None:
        mid_bufs = 1 if CK == 512 else 2
    if tick_order is None:
        tick_order = "acb" if CK == 512 else "abc"
    psum_bufs = tuple(psum_bufs)
    key = ("nc4", mid_bf16, CK, pin_bufs, mid_bufs, psum_bufs, perm_engines,
           io_eng_name, tick_order, sw_pipe, repeat, skip_perm, skip_store)
    if key in _NC_CACHE:
        return _NC_CACHE[key]

    import concourse.bass as bass  # noqa: F401
    import concourse.mybir as mybir
    import concourse.tile as tile
    from concourse import bacc

    f32 = mybir.dt.float32
    bf16 = mybir.dt.bfloat16
    mid_dt = bf16 if mid_bf16 else f32
    Identity = mybir.ActivationFunctionType.Identity

    nc = bacc.Bacc("TRN2", target_bir_lowering=False, debug=False)

    xt_d = nc.dram_tensor(
        "xt", [NCH, 128, 32, CK], mid_dt, kind="ExternalInput"
    )
    f_d = nc.dram_tensor("fw", [128, 512], mid_dt, kind="ExternalInput")
    w2_d = nc.dram_tensor("w2", [128, 4096], mid_dt, kind="ExternalInput")
    g_d = nc.dram_tensor("gw", [128, 512], mid_dt, kind="ExternalInput")
    bias_d = nc.dram_tensor("biasfm", [128, 32], f32, kind="ExternalInput")
    y_d = nc.dram_tensor(
        "y", [NCH, 128, 32, CK], mid_dt, kind="ExternalOutput"
    )

    perm_n = [0]

    MIDW = 32 * CK                    # mid tile width (elements per partition)
    n_store = MIDW // 4096            # ysb store pieces per chunk (2 or 4)
    reg_per_store = 4096 // CK        # (2j+bh) regions per ysb piece
    ck_per_bank = 512 // CK           # chunk-cols per PSUM bank (4, 2 or 1)
    jpb = max(1, ck_per_bank // 2)    # j's per PSUM bank in stage C
    jg = max(1, psum_bufs[2] * ck_per_bank // 2)   # j's in flight in stage C

    def perm_eng():
        engs = {
            "ss": (nc.sync, nc.scalar),
            "3way": (nc.sync, nc.scalar, nc.gpsimd),
            "sg": (nc.sync, nc.gpsimd),
            "s": (nc.sync,),
            "gp": (nc.gpsimd,),
        }[perm_engines]
        e = engs[perm_n[0] % len(engs)]
        perm_n[0] += 1
        return e

    io_engine = {"gpsimd": "gpsimd", "scalar": "scalar", "sync": "sync"}[io_eng_name]

    with tile.TileContext(nc) as tc:
        with (
            tc.tile_pool(name="wpool", bufs=1) as wpool,
            tc.tile_pool(name="pin", bufs=pin_bufs) as pin,
            tc.tile_pool(name="pt1", bufs=mid_bufs) as pt1,
            tc.tile_pool(name="pt1p", bufs=mid_bufs) as pt1p,
            tc.tile_pool(name="pt2", bufs=mid_bufs) as pt2,
            tc.tile_pool(name="pt2p", bufs=mid_bufs) as pt2p,
            tc.tile_pool(name="pysb", bufs=2) as pysb,
            tc.tile_pool(name="psA", bufs=psum_bufs[0], space="PSUM") as psA,
            tc.tile_pool(name="psB", bufs=psum_bufs[1], space="PSUM") as psB,
            tc.tile_pool(name="psC", bufs=psum_bufs[2], space="PSUM") as psC,
        ):
            f_sb = wpool.tile([128, 512], mid_dt, tag="fw")
            w2_sb = wpool.tile([128, 4096], mid_dt, tag="w2")
            g_sb = wpool.tile([128, 512], mid_dt, tag="gw")
            bias_sb = wpool.tile([128, 32], f32, tag="biasfm")
            nc.sync.dma_start(out=f_sb[:], in_=f_d[:])
            nc.sync.dma_start(out=w2_sb[:], in_=w2_d[:])
            nc.sync.dma_start(out=g_sb[:], in_=g_d[:])
            nc.sync.dma_start(out=bias_sb[:], in_=bias_d[:])

            evac_n = [0]

            def evac(dst, srcp):
                if evac_n[0] % 2 == 0:
                    nc.vector.tensor_copy(dst, srcp)
                else:
                    nc.scalar.copy(dst, srcp)
                evac_n[0] += 1

            def evac_bias(dst, srcp, bias_ap):
                if evac_n[0] % 2 == 0:
                    nc.vector.tensor_scalar_add(dst, srcp, bias_ap)
                else:
                    nc.scalar.activation(dst, srcp, Identity, bias=bias_ap)
                evac_n[0] += 1

            def io(eng_name):
                return getattr(nc, eng_name)

            xts_t = {}   # (ci, piece) -> tile
            t1p_t = {}
            t2p_t = {}

            def do_load(ci, piece=None):
                pieces = range(n_pieces) if piece is None else [piece]
                for h in pieces:
                    xts = pin.tile([128, pin_w], mid_dt, tag="pin")
                    fpp = 32 // n_pieces   # f-blocks per piece
                    io(io_engine).dma_start(
                        out=xts[:].rearrange("p (f t) -> p f t", f=fpp),
                        in_=xt_d[ci % NCH, :, h * fpp: (h + 1) * fpp],
                    )
                    xts_t[(ci, h)] = xts

            def do_A(ci, prefetch=None):
                # stage A: rfft per in-block; out t1[k, (16ch+i)*CK + t]
                t1 = pt1.tile([128, MIDW], mid_dt, tag="t1")
                fpp = 32 // n_pieces
                loaded = set()
                # ii-major so load piece h is released after its ii range
                for iq in range(0, NB, ck_per_bank):
                    for ch in range(2):
                        ps = psA.tile([128, 512], f32, tag="psA")
                        n_mm = 0
                        for ii in range(iq, iq + ck_per_bank):
                            for bh in range(2):
                                f = 2 * ii + bh
                                xts = xts_t[(ci, f // fpp)]
                                off = (ii - iq) * CK
                                nc.tensor.matmul(
                                    ps[:, off: off + CK],
                                    f_sb[:, bh * 256 + ch * 128:
                                         bh * 256 + ch * 128 + 128],
                                    xts[:, (f % fpp) * CK: (f % fpp) * CK + CK],
                                    start=(n_mm == 0),
                                    stop=(n_mm == 2 * ck_per_bank - 1),
                                )
                                n_mm += 1
                        q1 = 16 * ch + iq
                        evac(t1[:, q1 * CK: q1 * CK + ck_per_bank * CK], ps[:])
                    if prefetch is not None and n_pieces > 1:
                        # piece h of next chunk, once piece h of this one is done
                        h_done = (iq + ck_per_bank) * 2 // fpp - 1
                        if 0 <= h_done < n_pieces and h_done not in loaded:
                            loaded.add(h_done)
                            do_load(prefetch, piece=h_done)
                for h in range(n_pieces):
                    xts_t.pop((ci, h), None)
                if prefetch is not None and n_pieces == 1:
                    do_load(prefetch)
                # permute A->B: t1p[32s+q, g, t] = t1[4g+s, q, t]
                if skip_perm:
                    t1p_t[ci] = t1
                    return
                t1p = pt1p.tile([128, MIDW], mid_dt, tag="t1p")
                t1v = t1[:].rearrange("p (q t) -> p q t", t=CK)
                for g in range(32):
                    perm_eng().dma_start(
                        out=t1p[:, g * CK: g * CK + CK],
                        in_=t1v[4 * g: 4 * g + 4],
                    )
                t1p_t[ci] = t1p

            def do_B(ci):
                # stage B: per-slot-group mixing; out t2[(32s+16c+j), g*CK+t]
                t1p = t1p_t.pop(ci)
                t2 = pt2.tile([128, MIDW], mid_dt, tag="t2")
                for g0 in range(0, 32, ck_per_bank):
                    ps = psB.tile([128, 512], f32, tag="psB")
                    for gg in range(g0, g0 + ck_per_bank):
                        off = (gg - g0) * CK
                        nc.tensor.matmul(
                            ps[:, off: off + CK],
                            w2_sb[:, gg * 128: gg * 128 + 128],
                            t1p[:, gg * CK: gg * CK + CK],
                            start=(gg == g0),
                            stop=(gg == g0 + ck_per_bank - 1),
                        )
                    evac(t2[:, g0 * CK: g0 * CK + ck_per_bank * CK], ps[:])
                # permute B->C: t2p[4g+s, 16c+j, t] = t2[32s+16c+j, g, t]
                if skip_perm:
                    t2p_t[ci] = t2
                    return
                t2p = pt2p.tile([128, MIDW], mid_dt, tag="t2p")
                t2pv = t2p[:].rearrange("p (q t) -> p q t", t=CK)
                for g in range(32):
                    perm_eng().dma_start(
                        out=t2pv[4 * g: 4 * g + 4],
                        in_=t2[:, g * CK: g * CK + CK],
                    )
                t2p_t[ci] = t2p

            def do_C(ci):
                # stage C: irfft, feature-major; G slices stationary.
                # region q = 2j+bh of width CK; ysb pieces of 4096 cols
                t2p = t2p_t.pop(ci)
                for piece in range(n_store):
                    ysb = pysb.tile([128, 4096], mid_dt, tag="ysb")
                    q0 = piece * reg_per_store
                    j0, j1 = q0 // 2, (q0 + reg_per_store) // 2
                    for jq in range(j0, j1, jg):
                        pss = []
                        for _ in range(jg * 2 // ck_per_bank):
                            psc = psC.tile([128, 512], f32, tag="psC")
                            pss.append(psc)

                        def bank(jj, bh):
                            idx = (jj - jq) * 2 + bh
                            return pss[idx // ck_per_bank], (idx % ck_per_bank) * CK

                        for bh in range(2):
                            for ch in range(2):
                                for jj in range(jq, jq + jg):
                                    ps, off = bank(jj, bh)
                                    nc.tensor.matmul(
                                        ps[:, off: off + CK],
                                        g_sb[:, ch * 256 + bh * 128:
                                             ch * 256 + bh * 128 + 128],
                                        t2p[:, (16 * ch + jj) * CK:
                                            (16 * ch + jj) * CK + CK],
                                        start=(ch == 0 and (bh == 0 or ck_per_bank == 1)),
                                        stop=(ch == 1 and (bh == 1 or ck_per_bank == 1)),
                                    )
                        for jj in range(jq, jq + jg):
                            for bh in range(2):
                                ps, off = bank(jj, bh)
                                q = 2 * jj + bh
                                evac_bias(
                                    ysb[:, (q - q0) * CK: (q - q0) * CK + CK],
                                    ps[:, off: off + CK],
                                    bias_sb[:, q: q + 1],
                                )
                    if skip_store:
                        continue
                    io(io_engine).dma_start(
                        out=y_d[ci % NCH, :,
                                piece * reg_per_store: (piece + 1) * reg_per_store],
                        in_=ysb[:].rearrange("p (q t) -> p q t", t=CK),
                    )

            NT = NCH * repeat
            do_load(0)
            if NT > 1 and n_pieces == 1:
                do_load(1)
            def _pref(t):
                nxt = t + (2 if n_pieces == 1 else 1)
                return nxt if nxt < NT else None

            if sw_pipe:
                for t in range(NT + 2):
                    order = {
                        "acb": ("a", "c", "b"),
                        "abc": ("a", "b", "c"),
                        "bac": ("b", "a", "c"),
                        "bca": ("b", "c", "a"),
                    }[tick_order]
                    for st in order:
                        if st == "a" and t < NT:
                            do_A(t, prefetch=_pref(t))
                        elif st == "b" and 0 <= t - 1 < NT:
                            do_B(t - 1)
                        elif st == "c" and 0 <= t - 2 < NT:
                            do_C(t - 2)
            else:
                for t in range(NT):
                    do_A(t, prefetch=_pref(t))
                    do_B(t)
                    do_C(t)

    nc.compile()
    _NC_CACHE[key] = nc
    return nc


def prepare_inputs(x, c, bias, chunk=CHUNK):
    """Host-side prep: shard + pack x, build weights. Returns per-core in_maps."""
    batch, seq, in_f = x.shape
    n_tok = batch * seq
    nch = TOK_PER_CORE // chunk
    xf = np.ascontiguousarray(x.reshape(n_tok, in_f).astype(np.float32))

    f_host, w2_host, g_host = _build_weights(np.asarray(c, dtype=np.float32))
    bias = np.asarray(bias, dtype=np.float32)
    # biasfm[p, 2j+bh] = bias[j*256 + bh*128 + p]
    biasfm = np.ascontiguousarray(
        bias.reshape(NB, 2, 128).transpose(2, 0, 1).reshape(128, 32)
    )
    if MID_BF16:
        f_host = f_host.astype(_BF16)
        w2_host = w2_host.astype(_BF16)
        g_host = g_host.astype(_BF16)

    in_maps = []
    for core in range(N_CORES):
        shard = xf[core * TOK_PER_CORE:(core + 1) * TOK_PER_CORE]  # [1024, 4096]
        # xt[ci, p, f, t] = shard[ci*chunk + t, 128*f + p]
        xt = np.ascontiguousarray(
            shard.reshape(nch, chunk, 32, 128).transpose(0, 3, 2, 1)
        )
        if MID_BF16:
            xt = xt.astype(_BF16)
        in_maps.append(
            {
                "xt": xt,
                "fw": f_host,
                "w2": w2_host,
                "gw": g_host,
                "biasfm": biasfm,
            }
        )
    return in_maps


def postprocess(y_cores, chunk=CHUNK):
    """y_cores: list of per-core y arrays [NCH, 128, 32, chunk] ->
    full [8192, 4096] float32 (token-major)."""
    nch = TOK_PER_CORE // chunk
    outs = []
    for yc in y_cores:
        yc = np.asarray(yc, dtype=np.float32).reshape(nch, 128, NB, 2, chunk)
        # y[ci*chunk+t, j*256+bh*128+p] = yc[ci, p, j, bh, t]
        outs.append(
            yc.transpose(0, 4, 2, 3, 1).reshape(TOK_PER_CORE, IN_F)
        )
    return np.concatenate(outs, axis=0)


def kernel(x: np.ndarray, c: np.ndarray, bias: np.ndarray) -> np.ndarray:
    from concourse.bass_utils import run_bass_kernel_spmd

    batch, seq, in_f = x.shape
    in_maps = prepare_inputs(x, c, bias)
    nc = _build_module(mid_bf16=MID_BF16)
    res = run_bass_kernel_spmd(nc, in_maps, core_ids=list(range(N_CORES)))
    y = postprocess([r["y"] for r in res.results])
    return y.reshape(batch, seq, in_f).astype(x.dtype)
